# revision 63
# baseline (speedup 1.0000x reference)
"""CTGRU forward kernel for 8 trn2 NeuronCores (data-parallel over batch).

Layout on device (per core, local batch BL=512):
  - All per-step tensors live as [U_partitions, m*BL + b] ("layout C", m-major
    free dim), so the hidden state h comes out of the m-reduction already in
    the [U, B] orientation the next step's matmuls need as their moving
    operand -> zero transposes inside the recurrence.
  - softmax(-(z - LN_TAU)^2) is computed as Derivative_Erf(z + (b - LN_TAU))
    = (2/sqrt(pi)) * exp(-d^2); the constant cancels in the normalization.
  - DECAY[0] == 0 exactly, so h_hat[..., m=0] is identically zero: the state
    stores only m=1..7 (7 slices) and all elementwise work skips m=0.
  - All DVE elementwise/reduction traffic is bf16 with step-1 access so the
    DVE runs in its 2x packed mode (fp32 tensor_tensor is 1x); tree-reduce
    temps, gates, and the y matmul operands are bf16.
"""

import contextlib
import ctypes
import sys
import types

import numpy as np

B, T, F, U, M = 4096, 16, 256, 512, 8
N_CORES = 8
BL = B // N_CORES  # 512
KT = (F + U) // 128  # 6 K-tiles over the fused dim
UT = U // 128  # 4 u-tiles

LN_TAU = (np.arange(M, dtype=np.float32) * (0.5 * np.log(10.0))).astype(np.float32)
DECAY = np.exp(-0.04 / (LN_TAU + 1e-7)).astype(np.float32)  # DECAY[0] == 0.0


def _install_axon_hooks_shim():
    """Make `antenv.axon_hooks` importable when the image lacks it, so
    BASS_TRACE-triggered profiling in run_bass_kernel_spmd can't crash us."""
    name = "antenv.axon_hooks"
    if name in sys.modules:
        return
    so_path = "/opt/axon/libaxon_pjrt.so"

    def _build_hook():
        try:
            lib = ctypes.CDLL(so_path)
        except OSError:
            return None
        if not hasattr(lib, "axon_start_nrt_profile"):
            return None
        lib.axon_start_nrt_profile.argtypes = [
            ctypes.POINTER(ctypes.c_int64),
            ctypes.c_size_t,
        ]
        lib.axon_start_nrt_profile.restype = ctypes.c_int64
        lib.axon_stop_nrt_profile.argtypes = [ctypes.c_char_p]
        lib.axon_stop_nrt_profile.restype = ctypes.c_int64

        @contextlib.contextmanager
        def _hook(output_dir, device_ids):
            import jax

            jax.devices()
            if device_ids:
                ids = (ctypes.c_int64 * len(device_ids))(*device_ids)
                rc = lib.axon_start_nrt_profile(ids, len(device_ids))
            else:
                rc = lib.axon_start_nrt_profile(None, 0)
            if rc != 0:
                raise RuntimeError(f"axon_start_nrt_profile rc={rc}")
            try:
                yield
            finally:
                n = lib.axon_stop_nrt_profile(str(output_dir).encode())
                print(f"profile: {n} file(s) written to {output_dir}", file=sys.stderr)

        return _hook

    mod = types.ModuleType(name)
    holder = [_build_hook()]
    mod.get_axon_ntff_profile_hook = lambda: holder[0]
    mod.set_axon_ntff_profile_hook = lambda h: holder.__setitem__(0, h)
    sys.modules[name] = mod
    try:
        import antenv

        antenv.axon_hooks = mod
    except ImportError:
        pass


_program_cache = {}


def _build_program():
    if "nc" in _program_cache:
        return _program_cache["nc"]

    import concourse.bass as bass
    import concourse.tile as tile
    from concourse import bacc, mybir

    f32 = mybir.dt.float32
    bf16 = mybir.dt.bfloat16
    fp8 = mybir.dt.float8e4
    AF = mybir.ActivationFunctionType
    ALU = mybir.AluOpType
    DR = mybir.MatmulPerfMode.DoubleRow

    nc = bacc.Bacc("TRN2", target_bir_lowering=False, debug=False)

    WCH = 2 * KT * 128  # ws DMA chunk: 2 m-slices of one u-tile

    xt_d = nc.dram_tensor("xt", [T, 128, 2 * BL], bf16, kind="ExternalInput").ap()
    xt8_d = nc.dram_tensor("xt8", [T, 128, 2 * BL], fp8, kind="ExternalInput").ap()
    wr_d = nc.dram_tensor("wr", [128, M * UT * KT * 128], fp8, kind="ExternalInput").ap()
    # s-phase W: x-part (k-tiles 0,1) bf16 streamed; h-part (k-tiles 2..5)
    # fp8, resident, consumed as DoubleRow pairs against hT8
    wsx_d = nc.dram_tensor("wsx", [128, UT * M * 2 * 128], bf16, kind="ExternalInput").ap()
    ws8_d = nc.dram_tensor("ws8", [128, UT * M * 4 * 128], fp8, kind="ExternalInput").ap()
    wq_d = nc.dram_tensor("wq", [128, UT * KT * 128], bf16, kind="ExternalInput").ap()
    wo_d = nc.dram_tensor("wo", [128, UT * 3], bf16, kind="ExternalInput").ap()
    br_d = nc.dram_tensor("biasr", [128, UT * M], f32, kind="ExternalInput").ap()
    bs_d = nc.dram_tensor("biass", [128, UT * M], f32, kind="ExternalInput").ap()
    bq_d = nc.dram_tensor("biasq", [128, UT], f32, kind="ExternalInput").ap()
    y_d = nc.dram_tensor("y", [T, 3, BL], f32, kind="ExternalOutput").ap()

    with tile.TileContext(nc) as tc, contextlib.ExitStack() as ctx:
        const = ctx.enter_context(tc.tile_pool(name="const", bufs=1))
        state = ctx.enter_context(tc.tile_pool(name="state", bufs=1))
        wsp = ctx.enter_context(tc.tile_pool(name="wsp", bufs=2))
        rtp = ctx.enter_context(tc.tile_pool(name="rtp", bufs=2))
        grt = ctx.enter_context(tc.tile_pool(name="grt", bufs=2))
        xp = ctx.enter_context(tc.tile_pool(name="xp", bufs=2))
        ep = ctx.enter_context(tc.tile_pool(name="ep", bufs=2))
        tp = ctx.enter_context(tc.tile_pool(name="tp", bufs=2))
        tqp = ctx.enter_context(tc.tile_pool(name="tqp", bufs=2))
        wpp = ctx.enter_context(tc.tile_pool(name="wpp", bufs=2))
        sp = ctx.enter_context(tc.tile_pool(name="sp", bufs=2))
        stp = ctx.enter_context(tc.tile_pool(name="stp", bufs=2))
        pmm = ctx.enter_context(tc.tile_pool(name="pmm", bufs=5, space="PSUM"))
        pq = ctx.enter_context(tc.tile_pool(name="pq", bufs=2, space="PSUM"))
        py = ctx.enter_context(tc.tile_pool(name="py", bufs=1, space="PSUM"))

        # ---- weight / bias preload (wr/ws/wq are u-major: (u, m, k, c)) ----
        wr_sb = const.tile([128, M * UT * KT * 128], fp8, name="wr_sb")
        for u in range(UT):
            sl = slice(u * M * KT * 128, (u + 1) * M * KT * 128)
            nc.sync.dma_start(wr_sb[:, sl], wr_d[:, sl])
        ws8_sb = const.tile([128, UT * M * 4 * 128], fp8, name="ws8_sb")
        for u in range(UT):
            sl = slice(u * M * 4 * 128, (u + 1) * M * 4 * 128)
            nc.sync.dma_start(ws8_sb[:, sl], ws8_d[:, sl])
        # DECAY broadcast tile (m-major slices) for the single fused decay mult
        d_tile = const.tile([128, 7 * BL], bf16, name="d_tile")
        for mi in range(7):
            nc.gpsimd.memset(d_tile[:, mi * BL : (mi + 1) * BL], float(DECAY[mi + 1]))
        wq_sb = const.tile([128, UT * KT * 128], bf16, name="wq_sb")
        nc.sync.dma_start(wq_sb[:], wq_d[:])
        wo_sb = const.tile([128, UT * 3], bf16, name="wo_sb")
        nc.sync.dma_start(wo_sb[:], wo_d[:])
        br_sb = const.tile([128, UT * M], f32, name="br_sb")
        nc.sync.dma_start(br_sb[:], br_d[:])
        bs_sb = const.tile([128, UT * M], f32, name="bs_sb")
        nc.sync.dma_start(bs_sb[:], bs_d[:])
        bq_sb = const.tile([128, UT], f32, name="bq_sb")
        nc.sync.dma_start(bq_sb[:], bq_d[:])

        # h_hat state: m-slices 1..7 only (slice 0 is identically zero)
        hhat = [
            state.tile([128, 7 * BL], bf16, name=f"hhat{u}", tag=f"hhat{u}")
            for u in range(UT)
        ]

        hT = None  # [128, UT*BL] bf16, h(t) transposed — rhs k-tiles for h-part
        hT8 = None  # fp8 copy of hT for the DoubleRow r-phase matmuls

        def tree_reduce(src, n_m, out, eng=None, pairwise=True):
            """out[128,BL] = sum over n_m contiguous BL-slices of src (bf16
            temps so the DVE runs its 2x packed mode; `out` dtype may be f32,
            in which case only the final add runs 1x). GpSimd trees use their
            own temp pool so the two engines never share rotating buffers."""
            if eng is None:
                eng = nc.vector
            pool = rtp if eng is nc.vector else grt
            t1 = pool.tile([128, (4 if n_m == 8 else 3) * BL], bf16, name="rt1", tag="rt1")
            t2 = pool.tile([128, 2 * BL], bf16, name="rt2", tag="rt2")
            if n_m == 8:
                if eng is nc.vector and pairwise:
                    # pairwise level-1 so each add can fire as soon as its two
                    # activation slices land (better DVE/act pipelining)
                    for i in range(4):
                        eng.tensor_add(
                            t1[:, i * BL : (i + 1) * BL],
                            src[:, 2 * i * BL : (2 * i + 1) * BL],
                            src[:, (2 * i + 1) * BL : (2 * i + 2) * BL],
                        )
                    eng.tensor_add(t2[:], t1[:, : 2 * BL], t1[:, 2 * BL :])
                    eng.tensor_add(out[:], t2[:, :BL], t2[:, BL:])
                else:
                    eng.tensor_add(t1[:], src[:, : 4 * BL], src[:, 4 * BL :])
                    eng.tensor_add(t2[:], t1[:, : 2 * BL], t1[:, 2 * BL :])
                    eng.tensor_add(out[:], t2[:, :BL], t2[:, BL:])
            else:  # 7 slices: (0..2)+(4..6), then pairs, + slice 3
                eng.tensor_add(
                    t1[:, : 3 * BL], src[:, : 3 * BL], src[:, 4 * BL : 7 * BL]
                )
                eng.tensor_add(t2[:, :BL], t1[:, :BL], t1[:, BL : 2 * BL])
                eng.tensor_add(
                    t2[:, BL : 2 * BL], t1[:, 2 * BL : 3 * BL], src[:, 3 * BL : 4 * BL]
                )
                eng.tensor_add(out[:], t2[:, :BL], t2[:, BL : 2 * BL])

        def mm_group(ps, w_sb, base, rhs_x, rhs_h, with_h):
            """Accumulate the K=768 fused matmul into psum `ps`."""
            nc.tensor.matmul(
                ps[:], w_sb[:, base : base + 128], rhs_x[:, 0:BL],
                start=True, stop=False,
            )
            nc.tensor.matmul(
                ps[:], w_sb[:, base + 128 : base + 256], rhs_x[:, BL : 2 * BL],
                start=False, stop=not with_h,
            )
            if with_h:
                for k in range(2, KT):
                    nc.tensor.matmul(
                        ps[:],
                        w_sb[:, base + k * 128 : base + (k + 1) * 128],
                        rhs_h[:, (k - 2) * BL : (k - 1) * BL],
                        start=False, stop=(k == KT - 1),
                    )

        def pair(ap, n):
            return ap.rearrange("p (two f) -> p two f", two=2) if n else ap

        def mm_group8(ps, w_sb, base, rhs_x8, rhs_h8):
            """K=768 fused matmul as 3 fp8 DoubleRow matmuls (k-tile pairs)."""
            nc.tensor.matmul(
                ps[:], pair(w_sb[:, base : base + 256], 1),
                pair(rhs_x8[:, : 2 * BL], 1),
                start=True, stop=False, perf_mode=DR,
            )
            nc.tensor.matmul(
                ps[:], pair(w_sb[:, base + 256 : base + 512], 1),
                pair(rhs_h8[:, : 2 * BL], 1),
                start=False, stop=False, perf_mode=DR,
            )
            nc.tensor.matmul(
                ps[:], pair(w_sb[:, base + 512 : base + 768], 1),
                pair(rhs_h8[:, 2 * BL : 4 * BL], 1),
                start=False, stop=True, perf_mode=DR,
            )

        for t in range(T):
            xt_t = xp.tile([128, 2 * BL], bf16, name="xt_t")
            nc.sync.dma_start(xt_t[:], xt_d[t])
            xt8_t = xp.tile([128, 2 * BL], fp8, name="xt8_t")
            nc.sync.dma_start(xt8_t[:], xt8_d[t])

            # ---------------- r phase (t=0: h_hat==0 makes r irrelevant) ----
            if t > 0:
                rh_bf = stp.tile([128, UT * BL], bf16, name="rh_bf", bufs=1)
                for u in range(UT):
                    e_r = ep.tile([128, M * BL], bf16, name="e_t", tag="e_t")
                    for m in range(M):
                        ps = pmm.tile([128, BL], f32, name="ps_mm", tag="ps_mm")
                        base = (u * M + m) * KT * 128
                        mm_group8(ps, wr_sb, base, xt8_t, hT8)
                        nc.scalar.activation(
                            e_r[:, m * BL : (m + 1) * BL], ps[:],
                            AF.Derivative_Erf,
                            bias=br_sb[:, u * M + m : u * M + m + 1],
                        )
                    denr = sp.tile([128, BL], f32, name="den", tag="den")
                    tree_reduce(e_r, 8, denr)
                    cr = sp.tile([128, BL], f32, name="crec", tag="crec")
                    nc.vector.reciprocal_approx_fast(out=cr[:], in_=denr[:])
                    cr_bf = sp.tile([128, BL], bf16, name="crb", tag="crb")
                    nc.scalar.copy(cr_bf[:], cr[:])
                    eh = tp.tile([128, 7 * BL], bf16, name="ehr", tag="ehr")
                    nc.vector.tensor_mul(eh[:], e_r[:, BL:], hhat[u][:])
                    rhn = sp.tile([128, BL], bf16, name="rhn", tag="rhn")
                    tree_reduce(eh, 7, rhn)
                    nc.vector.tensor_mul(
                        rh_bf[:, u * BL : (u + 1) * BL], rhn[:], cr_bf[:]
                    )

            # ------- s phase matmuls / elementwise, interleaved with q -----
            # PE order: [r], s_mm(0), q, s_mm(1), s_el(0), s_mm(2), s_el(1),
            # ... so PE never stalls waiting for the r-phase elementwise tail.
            def s_mm(u):
                e_s = ep.tile([128, M * BL], bf16, name="e_t", tag="e_t")
                wschs = []
                for half in range(2):
                    wsch = wsp.tile([128, M * 128], bf16, name="wsch")
                    base = (u * M + half * (M // 2)) * 2 * 128
                    nc.sync.dma_start(wsch[:], wsx_d[:, base : base + M * 128])
                    wschs.append(wsch)
                for m in range(M):
                    ps = pmm.tile([128, BL], f32, name="ps_mm", tag="ps_mm")
                    wsch = wschs[m // (M // 2)]
                    xb = (m % (M // 2)) * 256
                    nc.tensor.matmul(
                        ps[:], wsch[:, xb : xb + 128], xt_t[:, 0:BL],
                        start=True, stop=False,
                    )
                    nc.tensor.matmul(
                        ps[:], wsch[:, xb + 128 : xb + 256], xt_t[:, BL : 2 * BL],
                        start=False, stop=(t == 0),
                    )
                    if t > 0:
                        hb = (u * M + m) * 4 * 128
                        nc.tensor.matmul(
                            ps[:], pair(ws8_sb[:, hb : hb + 256], 1),
                            pair(hT8[:, : 2 * BL], 1),
                            start=False, stop=False, perf_mode=DR,
                        )
                        nc.tensor.matmul(
                            ps[:], pair(ws8_sb[:, hb + 256 : hb + 512], 1),
                            pair(hT8[:, 2 * BL : 4 * BL], 1),
                            start=False, stop=True, perf_mode=DR,
                        )
                    nc.scalar.activation(
                        e_s[:, m * BL : (m + 1) * BL], ps[:],
                        AF.Derivative_Erf,
                        bias=bs_sb[:, u * M + m : u * M + m + 1],
                    )
                return e_s

            def s_el(u, e_s, q_bf, tq):
                dens = sp.tile([128, BL], f32, name="den", tag="den")
                tree_reduce(e_s, 8, dens)
                cs = sp.tile([128, BL], f32, name="crec", tag="crec")
                nc.vector.reciprocal_approx_fast(out=cs[:], in_=dens[:])
                cs_bf = sp.tile([128, BL], bf16, name="crb", tag="crb")
                nc.scalar.copy(cs_bf[:], cs[:])

                cs_v = cs_bf.unsqueeze(1).broadcast_to([128, 7, BL])
                wp = wpp.tile([128, 7 * BL], bf16, name="wp", tag="wp")
                wp_v = wp.rearrange("p (m b) -> p m b", m=7)
                es_v = e_s[:, BL:].rearrange("p (m b) -> p m b", m=7)
                # wp = e_s * (1/dens)  (gate, unnormalized-e trick)
                nc.vector.tensor_tensor(wp_v, es_v, cs_v, op=ALU.mult)
                if t > 0:
                    nc.vector.tensor_mul(wp[:], wp[:], tq[:])  # v = s*(q-h)
                    if u < 3:
                        # h+v in place on the state; the decay mult is
                        # deferred to scalar-engine per-slice muls at the end
                        # of the step (frees ~6us/step of DVE time)
                        nc.vector.tensor_add(hhat[u][:], hhat[u][:], wp[:])
                    else:
                        nc.vector.tensor_add(wp[:], hhat[u][:], wp[:])
                else:
                    q_v = (
                        q_bf[:, u * BL : (u + 1) * BL]
                        .unsqueeze(1)
                        .broadcast_to([128, 7, BL])
                    )
                    nc.vector.tensor_tensor(wp_v, wp_v, q_v, op=ALU.mult)
                if t == 0 or u == 3:
                    nc.vector.tensor_mul(hhat[u][:], wp[:], d_tile[:])
                    # u<3: h tree on gpsimd (slack until step end); u=3 is the
                    # tail into next step's matmuls -> fast DVE
                    tree_reduce(
                        hhat[u], 7, hT_new[:, u * BL : (u + 1) * BL],
                        eng=nc.gpsimd if u < 3 else nc.vector,
                    )
                    nc.scalar.copy(
                        hT8_new[:, u * BL : (u + 1) * BL],
                        hT_new[:, u * BL : (u + 1) * BL],
                    )

            e_tiles = {0: s_mm(0)}

            # ---------------- q phase ----------------
            q_bf = stp.tile([128, UT * BL], bf16, name="q_bf", bufs=1)
            for uq in range(UT):
                psq = pq.tile([128, BL], f32, name="ps_q", tag="ps_q")
                mm_group(psq, wq_sb, uq * KT * 128, xt_t, rh_bf if t > 0 else None, t > 0)
                nc.scalar.activation(
                    q_bf[:, uq * BL : (uq + 1) * BL], psq[:],
                    AF.Tanh, bias=bq_sb[:, uq : uq + 1],
                )

            # tq(u) = q - h_hat, hoisted out of the s_el chain so the v-mult
            # doesn't serialize behind the cs reciprocal path. Issued two at a
            # time (staggered) so the pool-WAR on tq buffers is satisfied by
            # the time each is issued instead of stalling the DVE queue.
            tqs = [None] * UT

            def make_tq(u):
                if t == 0:
                    return
                tq = tqp.tile([128, 7 * BL], bf16, name="tq", tag="tq")
                tq_v = tq.rearrange("p (m b) -> p m b", m=7)
                q_v = (
                    q_bf[:, u * BL : (u + 1) * BL]
                    .unsqueeze(1)
                    .broadcast_to([128, 7, BL])
                )
                hh_v = hhat[u].rearrange("p (m b) -> p m b", m=7)
                nc.vector.tensor_tensor(tq_v, q_v, hh_v, op=ALU.subtract)
                tqs[u] = tq

            for u in range(UT):
                make_tq(u)

            hT_new = stp.tile([128, UT * BL], bf16, name="hT")
            hT8_new = stp.tile([128, UT * BL], fp8, name="hT8")
            for u in range(UT):
                if t > 0 and u == 3:
                    # deferred decay for u=0..2 on the scalar engine: issued
                    # after all s acts (never blocks the psum drain), executed
                    # while the DVE works the u=3 tail
                    for ud in range(3):
                        for mi in range(7):
                            nc.scalar.mul(
                                hhat[ud][:, mi * BL : (mi + 1) * BL],
                                hhat[ud][:, mi * BL : (mi + 1) * BL],
                                float(DECAY[mi + 1]),
                            )
                s_el(u, e_tiles.pop(u), q_bf, tqs[u])
                if u + 1 < UT:
                    e_tiles[u + 1] = s_mm(u + 1)
            if t > 0:
                for ud in range(3):
                    tree_reduce(
                        hhat[ud], 7, hT_new[:, ud * BL : (ud + 1) * BL],
                        eng=nc.gpsimd,
                    )
                    nc.scalar.copy(
                        hT8_new[:, ud * BL : (ud + 1) * BL],
                        hT_new[:, ud * BL : (ud + 1) * BL],
                    )
            hT = hT_new
            hT8 = hT8_new

            # ---------------- y phase ----------------
            psy = py.tile([3, BL], f32, name="ps_y", tag="ps_y")
            for k in range(UT):
                nc.tensor.matmul(
                    psy[:], wo_sb[:, k * 3 : (k + 1) * 3],
                    hT[:, k * BL : (k + 1) * BL],
                    start=(k == 0), stop=(k == UT - 1),
                )
            y_sb = stp.tile([3, BL], f32, name="y_sb")
            nc.vector.tensor_copy(y_sb[:], psy[:])
            nc.sync.dma_start(y_d[t], y_sb[:])

    nc.compile()
    _program_cache["nc"] = nc
    return nc


def _prep_shared(W_r, b_r, W_q, b_q, W_s, b_s, W_out):
    import ml_dtypes

    bf = ml_dtypes.bfloat16

    def perm_w(w, dt):  # [768, 4096] -> [128, (u,m,k,c)]
        a = np.ascontiguousarray(w, np.float32).reshape(KT, 128, UT, 128, M)
        return np.ascontiguousarray(
            a.transpose(1, 2, 4, 0, 3).reshape(128, M * UT * KT * 128)
        ).astype(dt)

    wr = perm_w(W_r, ml_dtypes.float8_e4m3)
    a = (
        np.ascontiguousarray(W_s, np.float32)
        .reshape(KT, 128, UT, 128, M)
        .transpose(1, 2, 4, 0, 3)  # [128, UT, M, KT, 128]
    )
    wsx = np.ascontiguousarray(a[:, :, :, :2, :].reshape(128, UT * M * 2 * 128)).astype(bf)
    ws8 = np.ascontiguousarray(a[:, :, :, 2:, :].reshape(128, UT * M * 4 * 128)).astype(
        ml_dtypes.float8_e4m3
    )
    wq = np.ascontiguousarray(
        np.asarray(W_q, np.float32)
        .reshape(KT, 128, UT, 128)
        .transpose(1, 2, 0, 3)
        .reshape(128, UT * KT * 128)
    ).astype(bf)
    wo = np.ascontiguousarray(
        np.asarray(W_out, np.float32).reshape(UT, 128, 3).transpose(1, 0, 2).reshape(128, UT * 3)
    ).astype(bf)
    biasr = np.ascontiguousarray(
        (np.asarray(b_r, np.float32).reshape(UT, 128, M) - LN_TAU).transpose(1, 0, 2).reshape(128, UT * M)
    )
    biass = np.ascontiguousarray(
        (np.asarray(b_s, np.float32).reshape(UT, 128, M) - LN_TAU).transpose(1, 0, 2).reshape(128, UT * M)
    )
    biasq = np.ascontiguousarray(np.asarray(b_q, np.float32).reshape(UT, 128).T)
    return dict(
        wr=wr, wsx=wsx, ws8=ws8, wq=wq, wo=wo, biasr=biasr, biass=biass, biasq=biasq
    )


def kernel(x, W_r, b_r, W_q, b_q, W_s, b_s, W_out, b_out):
    _install_axon_hooks_shim()
    from concourse.bass_utils import run_bass_kernel_spmd

    import ml_dtypes

    bf = ml_dtypes.bfloat16

    nc = _build_program()
    shared = _prep_shared(W_r, b_r, W_q, b_q, W_s, b_s, W_out)

    x = np.asarray(x, np.float32)
    in_maps = []
    for c in range(N_CORES):
        xc = x[c * BL : (c + 1) * BL]  # [BL, T, F]
        xt = np.ascontiguousarray(
            xc.transpose(1, 2, 0).reshape(T, 2, 128, BL).transpose(0, 2, 1, 3).reshape(T, 128, 2 * BL)
        )
        in_maps.append(
            {
                "xt": xt.astype(bf),
                "xt8": xt.astype(ml_dtypes.float8_e4m3),
                **shared,
            }
        )

    try:
        res = run_bass_kernel_spmd(nc, in_maps, list(range(N_CORES)))
    except Exception:
        # device pool may be wedged from an earlier crash — reset and retry
        try:
            lib = ctypes.CDLL("/opt/axon/libaxon_pjrt.so")
            lib.axon_reset.restype = ctypes.c_int64
            lib.axon_reset()
        except OSError:
            pass
        res = run_bass_kernel_spmd(nc, in_maps, list(range(N_CORES)))
    _program_cache["last_result"] = res

    out = np.empty((B, T, 3), np.float32)
    for c in range(N_CORES):
        y = res.results[c]["y"]  # [T, 3, BL]
        out[c * BL : (c + 1) * BL] = y.transpose(2, 0, 1)
    return out + np.asarray(b_out, np.float32)
